# revision 18
# baseline (speedup 1.0000x reference)
"""Trainium2 Bass kernel for CenterWoParamMultiCosineSoftmaxLoss.

loss = mean_b sum_k softmax_k(2 - dst_bk) * dst_bk,
  dst_bk = 1 - <x_b/||x_b||, c_{l_b,k}/||c_{l_b,k}||>

Identities used:
  softmax(2 - dst) = softmax(s)     (shift invariance; s = cosine score)
  per_sample       = 1 - sum_k p_k s_k
  xT is pre-scaled by rnorm_b during the PSUM->SBUF pack copy, so the
  matmul emits normalized scores directly; Z and num come from one exp
  pass + segmented reduces.

Distribution: samples grouped by label on the host into 256-slot segments
(one class per segment), 12 segments per core, identical SPMD program on 8
cores. Pad slots are zero rows and contribute exactly 0.

Pipeline per core (slots=3072, 24 sub-chunks of 128 rows):
  - x DMAs land 2 segments at a time as [128, 4KB] lines (row pair 2p,2p+1
    per partition) - 4KB contiguous DRAM reads per partition line.
  - per sub-chunk: sum-of-squares (ACT Square+accum / DVE fused
    tensor_tensor_reduce, alternating), pair-batched rsqrt via ln/exp,
    4 fp32 PE transposes, scaled pack-copy (x * rnorm -> bf16 xT,
    ACT/DVE alternating), 4 bf16 accumulating score matmuls.
  - per group of 6 sub-chunks: one exp over [128,192], segmented Z and
    num reduces on DVE.
  - tail: t = num/Z batched, row reduce, PE ones-matmul, DMA out.
"""

import sys

for _p in ("/opt/trn_rl_repo", "/root/.axon_site/_ro/trn_rl_repo"):
    if _p not in sys.path:
        sys.path.append(_p)

import numpy as np

import concourse.bass as bass
import concourse.mybir as mybir
from concourse.tile import TileContext
from concourse.masks import make_identity
from concourse.bass_utils import run_bass_kernel_spmd
from concourse.vector_clock import ScopedClock

B, D, C, K = 16384, 512, 90, 32
NCORES = 8
SEGW = 256          # slots per segment (one class per segment), 2 chunks of 128
P = 128
DCH = D // P        # 4 contraction chunks
f32 = mybir.dt.float32
bf16 = mybir.dt.bfloat16
AF = mybir.ActivationFunctionType
ALU = mybir.AluOpType

_tile_patched = False


def _install_tile_patch():
    """This walrus build allows only one sem wait on TPB_CTRL-lowered
    instructions (Drain / sync-NoOp). Tile's tail drain attaches one wait per
    live processor clock; split them into a chain of single-wait NoOps."""
    global _tile_patched
    if _tile_patched:
        return
    _tile_patched = True

    def _drain_and_barrier(self, tick_clock, wait_clock):
        nc = self.nc
        probe = nc.sync.nop(nofuse=True)
        wait_clock.add_sem_waits(
            probe.ins, ScopedClock({None: tick_clock.global_clock})
        )
        si = probe.ins.sync_info
        if si is not None and len(si.on_wait) > 1:
            waits = list(si.on_wait)
            si.on_wait.clear()
            si.on_wait.append(waits[0])
            for w in waits[1:]:
                n2 = nc.sync.nop(nofuse=True)
                if n2.ins.sync_info is None:
                    n2.ins.sync_info = mybir.SyncInfo(on_wait=[w], on_update=[])
                else:
                    n2.ins.sync_info.on_wait.append(w)
        nc.sync.drain()
        nc.all_engine_barrier()
        assert self.sems is not None
        popped = nc._tile_sem_poison_stack.pop()
        assert popped is self._sem_poison
        nc.clear_and_free_semaphores(list(self.sems.allocated().values()))
        nc.all_engine_barrier()

    TileContext._drain_and_barrier = _drain_and_barrier


def _split_excess_waits(nc, max_waits=1):
    """This walrus build accepts at most one sem wait per instruction for
    several opcodes. Hoist excess waits onto single-wait NoOps emitted just
    before the instruction on the same engine (engine streams are serial, so
    semantics are preserved)."""
    n = 0
    for fn in nc.m.functions:
        for blk in fn.blocks:
            newl = []
            for inst in blk.instructions:
                si = getattr(inst, "sync_info", None)
                if si is not None and si.on_wait is not None and len(si.on_wait) > max_waits:
                    waits = list(si.on_wait)
                    keep = waits[-max_waits:]
                    extra = waits[:-max_waits]
                    si.on_wait.clear()
                    for w in keep:
                        si.on_wait.append(w)
                    for w in extra:
                        n += 1
                        newl.append(
                            mybir.InstNoOp(
                                name=f"{inst.name}-w{n}",
                                engine=inst.engine,
                                sync_info=mybir.SyncInfo(on_wait=[w], on_update=[]),
                                bass_nofuse=True,
                            )
                        )
                newl.append(inst)
            blk.instructions[:] = newl
    return nc


def build_bass(nseg: int, split_waits: bool = True):
    """One core's program: nseg segments of SEGW class-grouped sample slots."""
    _install_tile_patch()
    slots = nseg * SEGW
    nch = slots // P                  # sub-chunks of 128 rows (2 per segment)
    ck = nseg * K                     # center rows used
    ct = (ck + P - 1) // P            # center row tiles
    ckp = ct * P                      # padded center rows
    npair = nseg // 2                 # 2-segment DMA quanta
    odd = nseg % 2

    # softmax group = 6 sub-chunks -> one PSUM bank [128, 192] f32
    GRP = max(6, (nch + 3) // 4)
    ngrp = (nch + GRP - 1) // GRP

    nc = bass.Bass()
    xg = nc.dram_tensor("xg", [slots, D], f32, kind="ExternalInput")
    cent = nc.dram_tensor("cent", [ckp, D], f32, kind="ExternalInput")
    out = nc.dram_tensor("partial", [1, 1], f32, kind="ExternalOutput")

    with TileContext(nc) as tc:
        with (
            tc.tile_pool(name="const", bufs=1) as const_pool,
            tc.tile_pool(name="persist", bufs=1) as persist,
            tc.tile_pool(name="cin", bufs=1) as cin_pool,
            tc.tile_pool(name="cnb", bufs=4) as cnb_pool,
            tc.tile_pool(name="junk", bufs=6) as junk_pool,
            tc.tile_pool(name="esb", bufs=2) as esb_pool,
            tc.tile_pool(name="tp_ps", bufs=2, space="PSUM") as tp_psum,
            tc.tile_pool(name="sc_ps", bufs=1, space="PSUM") as sc_psum,
            tc.tile_pool(name="fin_ps", bufs=1, space="PSUM") as fin_psum,
        ):
            id_f32 = const_pool.tile([P, P], f32)
            make_identity(nc, id_f32[:])
            id_bf16 = const_pool.tile([P, P], bf16)
            make_identity(nc, id_bf16[:])
            ones = const_pool.tile([P, 1], f32)
            nc.gpsimd.memset(ones[:], 1.0)

            # persistent tensors
            xf = persist.tile([P, nch * D], f32)       # sub-chunk i at cols [i*D, +D)
            xfb = persist.tile([P, nch * D], bf16)     # bf16 cast of xf (gpsimd)
            xT = persist.tile([P, DCH * slots], bf16)  # d-chunk c at cols [c*slots, +slots)
            cnT = persist.tile([P, DCH * ckp], bf16)   # d-chunk c at cols [c*ckp, +ckp)
            ssq = persist.tile([P, nch], f32)          # sum_d x^2 per sub-chunk col
            rnorm = persist.tile([P, nch], f32)        # rsqrt(ss + eps)
            zsum = persist.tile([P, nch], f32)         # softmax denominators
            nums = persist.tile([P, nch], f32)         # sum_k e_k * s_k
            c_ssr = persist.tile([P, ct], f32)         # ss_c then +eps
            c_ln = persist.tile([P, ct], f32)
            c_rn = persist.tile([P, ct], f32)          # rsqrt(ss_c + eps)

            # ---- all DMAs issued up front on the sync queue: the ring
            # streams centers then x pairs back to back.
            cfs = []
            for t in range(ct):
                cf = cin_pool.tile([P, D], f32, tag=f"cin{t}")
                cfs.append(cf)
                nc.sync.dma_start(out=cf[:], in_=cent[t * P:(t + 1) * P, :])
            for q in range(npair):
                src = xg[2 * q * SEGW:(2 * q + 2) * SEGW, :].rearrange(
                    "(s p r) d -> p s r d", s=2, p=P, r=2
                )
                dst = xf[:, q * 4 * D:(q + 1) * 4 * D].rearrange(
                    "p (s r d) -> p s r d", s=2, r=2, d=D
                )
                nc.sync.dma_start(out=dst, in_=src)
            if odd:
                src = xg[(nseg - 1) * SEGW:nseg * SEGW, :].rearrange(
                    "(p r) d -> p r d", p=P, r=2
                )
                dst = xf[:, npair * 4 * D:(npair * 4 + 2) * D].rearrange(
                    "p (r d) -> p r d", r=2, d=D
                )
                nc.sync.dma_start(out=dst, in_=src)

            # ---- centers: row sum-of-squares, rsqrt, scaled bf16 copy,
            # transpose into cnT.
            for t in range(ct):
                cjunk = junk_pool.tile([P, D], f32, tag="junkA")
                nc.scalar.activation(
                    out=cjunk[:], in_=cfs[t][:], func=AF.Square,
                    accum_out=c_ssr[:, t:t + 1],
                )
            nc.vector.tensor_scalar_add(out=c_ssr[:], in0=c_ssr[:], scalar1=1e-12)
            nc.scalar.activation(out=c_ln[:], in_=c_ssr[:], func=AF.Ln)
            nc.scalar.activation(out=c_rn[:], in_=c_ln[:], func=AF.Exp, scale=-0.5)
            for t in range(ct):
                cb = cnb_pool.tile([P, D], bf16, tag="cnb")
                nc.scalar.activation(
                    out=cb[:], in_=cfs[t][:], func=AF.Copy, scale=c_rn[:, t:t + 1],
                )
                cps = tp_psum.tile([P, D], bf16, tag="ctp", bufs=1)
                for c in range(DCH):
                    nc.tensor.transpose(
                        cps[:, c * P:(c + 1) * P], cb[:, c * P:(c + 1) * P],
                        id_bf16[:],
                    )
                nc.vector.tensor_copy(
                    out=cnT[:].rearrange("p (c n) -> p c n", c=DCH)[
                        :, :, t * P:(t + 1) * P
                    ],
                    in_=cps[:].rearrange("p (c n) -> p c n", c=DCH),
                )

            # ---- x pipeline ----
            scps = []
            egrp = []
            for g in range(ngrp):
                scp_g = sc_psum.tile([P, GRP * K], f32, tag=f"scp{g}")
                scps.append(scp_g)
                e_g = esb_pool.tile([P, GRP * K], f32, tag=f"esb{g}", bufs=1)
                ssc_g = esb_pool.tile([P, GRP * K], f32, tag=f"ssc{g}", bufs=1)
                egrp.append((e_g, ssc_g))

            mv = persist.tile([P, 2 * nch], f32)   # (mean, var) for DVE-ss subs
            mv3 = mv[:].rearrange("p (i two) -> p i two", two=2)
            ssq3 = ssq[:].rearrange("p (i one) -> p i one", one=1)
            rn3 = rnorm[:].rearrange("p (i one) -> p i one", one=1)

            def quantum(subs):
                q = subs[0] // 4
                w = len(subs) * D
                # 0) bf16 cast of the whole quantum on the idle gpsimd
                nc.gpsimd.tensor_copy(
                    out=xfb[:, subs[0] * D: subs[0] * D + w],
                    in_=xf[:, subs[0] * D: subs[0] * D + w],
                )
                # 1) sum of squares per sub-chunk: 1 on ACT, rest on DVE
                h = 1
                for idx, i in enumerate(subs):
                    xfi = xfb[:, i * D:(i + 1) * D]
                    if idx < h:
                        ja = junk_pool.tile([P, D], bf16, tag="junkA")
                        nc.scalar.activation(
                            out=ja[:], in_=xfi, func=AF.Square,
                            accum_out=ssq[:, i:i + 1],
                        )
                    else:
                        bns = junk_pool.tile([P, 6], f32, tag="bns")
                        nc.vector.bn_stats(out=bns[:], in_=xfi)
                        nc.vector.bn_aggr(out=mv[:, 2 * i:2 * i + 2], in_=bns[:])
                # ss = D*(var + mean^2) for the DVE subs (contiguous)
                a, b = subs[h], subs[-1] + 1
                nc.vector.tensor_mul(
                    out=ssq3[:, a:b], in0=mv3[:, a:b, 0:1], in1=mv3[:, a:b, 0:1]
                )
                nc.vector.tensor_add(
                    out=ssq3[:, a:b], in0=ssq3[:, a:b], in1=mv3[:, a:b, 1:2]
                )
                nc.vector.tensor_scalar_mul(
                    out=ssq[:, a:b], in0=ssq[:, a:b], scalar1=float(D)
                )
                # 2) transpose (bf16), pack copy, score matmuls
                for idx, i in enumerate(subs):
                    tps = tp_psum.tile([P, D], bf16, tag="tp")
                    for c in range(DCH):
                        nc.tensor.transpose(
                            tps[:, c * P:(c + 1) * P],
                            xfb[:, i * D + c * P: i * D + (c + 1) * P],
                            id_bf16[:],
                        )
                    xt_dst = xT[:].rearrange("p (c n) -> p c n", c=DCH)[
                        :, :, i * P:(i + 1) * P
                    ]
                    tps_src = tps[:].rearrange("p (c n) -> p c n", c=DCH)
                    if idx % 4 == 3:
                        nc.scalar.activation(
                            out=xt_dst, in_=tps_src, func=AF.Copy,
                        )
                    else:
                        nc.vector.tensor_copy(out=xt_dst, in_=tps_src)
                    j = i // 2          # class/segment of this sub-chunk
                    g = i // GRP
                    sc = scps[g][:, (i - g * GRP) * K:(i - g * GRP + 1) * K]
                    for c in range(DCH):
                        nc.tensor.matmul(
                            sc,
                            xT[:, c * slots + i * P: c * slots + (i + 1) * P],
                            cnT[:, c * ckp + j * K: c * ckp + (j + 1) * K],
                            start=(c == 0),
                            stop=(c == DCH - 1),
                        )
                # 3) group softmax once a group's scores are complete:
                # rnorm batch (eps+ln+exp), ssc = s_raw * rnorm (broadcast),
                # e = exp(ssc), Z and num via segmented reduces.
                for i in subs:
                    g = i // GRP
                    if i != min((g + 1) * GRP, nch) - 1:
                        continue
                    c0, c1 = g * GRP, min((g + 1) * GRP, nch)
                    gw = c1 - c0
                    nc.vector.tensor_scalar_add(
                        out=ssq[:, c0:c1], in0=ssq[:, c0:c1], scalar1=1e-12
                    )
                    nc.scalar.activation(
                        out=rnorm[:, c0:c1], in_=ssq[:, c0:c1], func=AF.Ln
                    )
                    nc.scalar.activation(
                        out=rnorm[:, c0:c1], in_=rnorm[:, c0:c1], func=AF.Exp,
                        scale=-0.5,
                    )
                    ssc = egrp[g][1]
                    ssc3 = ssc[:].rearrange("p (i k) -> p i k", k=K)
                    nc.vector.tensor_mul(
                        out=ssc3[:, :gw],
                        in0=scps[g][:, :gw * K].rearrange(
                            "p (i k) -> p i k", k=K
                        ),
                        in1=rn3[:, c0:c1].broadcast_to((P, gw, K)),
                    )
                    e = egrp[g][0]
                    nc.scalar.activation(
                        out=e[:, :gw * K], in_=ssc[:, :gw * K], func=AF.Exp,
                    )
                    e3 = e[:].rearrange("p (i k) -> p i k", k=K)
                    nc.vector.tensor_reduce(
                        out=zsum[:, c0:c1], in_=e3[:, :gw],
                        axis=mybir.AxisListType.X, op=ALU.add,
                    )
                    jk = junk_pool.tile([P, GRP * K], f32, tag="jk")
                    nc.vector.tensor_mul(
                        out=jk[:, :gw * K], in0=e[:, :gw * K],
                        in1=ssc[:, :gw * K],
                    )
                    jk3 = jk[:].rearrange("p (i k) -> p i k", k=K)
                    nc.vector.tensor_reduce(
                        out=nums[:, c0:c1], in_=jk3[:, :gw],
                        axis=mybir.AxisListType.X, op=ALU.add,
                    )

            for q in range(npair):
                quantum([4 * q, 4 * q + 1, 4 * q + 2, 4 * q + 3])
            if odd:
                quantum([4 * npair, 4 * npair + 1])

            # ---- tail: t = num / Z, partial = sum over all slots ----
            nc.vector.reciprocal(out=zsum[:], in_=zsum[:])
            nc.vector.tensor_mul(out=nums[:], in0=nums[:], in1=zsum[:])
            red = persist.tile([P, 1], f32)
            nc.vector.tensor_reduce(
                out=red[:], in_=nums[:], axis=mybir.AxisListType.X, op=ALU.add,
            )
            fin = fin_psum.tile([1, 1], f32)
            nc.tensor.matmul(fin[:], red[:], ones[:], start=True, stop=True)
            osb = const_pool.tile([1, 1], f32)
            nc.scalar.copy(out=osb[:], in_=fin[:])
            nc.sync.dma_start(out=out[:], in_=osb[:])

    if split_waits:
        _split_excess_waits(nc)
    return nc


def _pack_segments(labels: np.ndarray):
    """Group sample indices by label into segments of <= SEGW, one class per
    segment; pad total segment count to a multiple of NCORES."""
    order = np.argsort(labels, kind="stable")
    sorted_lab = labels[order]
    cut = np.flatnonzero(np.diff(sorted_lab)) + 1
    starts = np.concatenate(([0], cut))
    ends = np.concatenate((cut, [len(labels)]))
    segs = []  # (class, sample_index_array)
    for s, e in zip(starts, ends):
        cls = int(sorted_lab[s])
        for o in range(s, e, SEGW):
            segs.append((cls, order[o:min(o + SEGW, e)]))
    while len(segs) % NCORES != 0:
        segs.append((0, np.empty(0, dtype=np.int64)))
    return segs


def kernel(x: np.ndarray, labels: np.ndarray, centers: np.ndarray) -> np.ndarray:
    x = np.ascontiguousarray(x, dtype=np.float32)
    labels = np.asarray(labels)
    centers = np.ascontiguousarray(centers, dtype=np.float32)
    nb, d = x.shape
    ncls, k, _ = centers.shape
    assert (nb, d, k) == (B, D, K)

    segs = _pack_segments(labels)
    nseg_total = len(segs)
    nseg = nseg_total // NCORES
    slots = nseg * SEGW
    ck = nseg * K
    ckp = ((ck + P - 1) // P) * P

    in_maps = []
    for core in range(NCORES):
        xg = np.zeros((slots, d), dtype=np.float32)
        cent = np.zeros((ckp, d), dtype=np.float32)
        for jj in range(nseg):
            cls, idx = segs[core * nseg + jj]
            if len(idx):
                xg[jj * SEGW: jj * SEGW + len(idx)] = x[idx]
            cent[jj * K:(jj + 1) * K] = centers[cls]
        in_maps.append({"xg": xg, "cent": cent})

    nc = build_bass(nseg)
    res = run_bass_kernel_spmd(nc, in_maps, core_ids=list(range(NCORES)))
    total = sum(float(r["partial"][0, 0]) for r in res.results)
    return np.float32(1.0 - total / nb)


# revision 19
# speedup vs baseline: 1.6584x; 1.6584x over previous
"""Trainium2 Bass kernel for CenterWoParamMultiCosineSoftmaxLoss.

loss = mean_b sum_k softmax_k(2 - dst_bk) * dst_bk,
  dst_bk = 1 - <x_b/||x_b||, c_{l_b,k}/||c_{l_b,k}||>

Identities used:
  softmax(2 - dst) = softmax(s)     (shift invariance; s = cosine score)
  per_sample       = 1 - sum_k p_k s_k
  xT is pre-scaled by rnorm_b during the PSUM->SBUF pack copy, so the
  matmul emits normalized scores directly; Z and num come from one exp
  pass + segmented reduces.

Distribution: samples grouped by label on the host into 256-slot segments
(one class per segment), 12 segments per core, identical SPMD program on 8
cores. Pad slots are zero rows and contribute exactly 0.

Pipeline per core (slots=3072, 24 sub-chunks of 128 rows):
  - x DMAs land 2 segments at a time as [128, 4KB] lines (row pair 2p,2p+1
    per partition) - 4KB contiguous DRAM reads per partition line.
  - per sub-chunk: sum-of-squares (ACT Square+accum / DVE fused
    tensor_tensor_reduce, alternating), pair-batched rsqrt via ln/exp,
    4 fp32 PE transposes, scaled pack-copy (x * rnorm -> bf16 xT,
    ACT/DVE alternating), 4 bf16 accumulating score matmuls.
  - per group of 6 sub-chunks: one exp over [128,192], segmented Z and
    num reduces on DVE.
  - tail: t = num/Z batched, row reduce, PE ones-matmul, DMA out.
"""

import sys

for _p in ("/opt/trn_rl_repo", "/root/.axon_site/_ro/trn_rl_repo"):
    if _p not in sys.path:
        sys.path.append(_p)

import numpy as np

import concourse.bass as bass
import concourse.mybir as mybir
from concourse.tile import TileContext
from concourse.masks import make_identity
from concourse.bass_utils import run_bass_kernel_spmd
from concourse.vector_clock import ScopedClock

B, D, C, K = 16384, 512, 90, 32
NCORES = 8
SEGW = 256          # slots per segment (one class per segment), 2 chunks of 128
P = 128
DCH = D // P        # 4 contraction chunks
f32 = mybir.dt.float32
bf16 = mybir.dt.bfloat16
AF = mybir.ActivationFunctionType
ALU = mybir.AluOpType

_tile_patched = False


def _install_tile_patch():
    """This walrus build allows only one sem wait on TPB_CTRL-lowered
    instructions (Drain / sync-NoOp). Tile's tail drain attaches one wait per
    live processor clock; split them into a chain of single-wait NoOps."""
    global _tile_patched
    if _tile_patched:
        return
    _tile_patched = True

    def _drain_and_barrier(self, tick_clock, wait_clock):
        nc = self.nc
        probe = nc.sync.nop(nofuse=True)
        wait_clock.add_sem_waits(
            probe.ins, ScopedClock({None: tick_clock.global_clock})
        )
        si = probe.ins.sync_info
        if si is not None and len(si.on_wait) > 1:
            waits = list(si.on_wait)
            si.on_wait.clear()
            si.on_wait.append(waits[0])
            for w in waits[1:]:
                n2 = nc.sync.nop(nofuse=True)
                if n2.ins.sync_info is None:
                    n2.ins.sync_info = mybir.SyncInfo(on_wait=[w], on_update=[])
                else:
                    n2.ins.sync_info.on_wait.append(w)
        nc.sync.drain()
        nc.all_engine_barrier()
        assert self.sems is not None
        popped = nc._tile_sem_poison_stack.pop()
        assert popped is self._sem_poison
        nc.clear_and_free_semaphores(list(self.sems.allocated().values()))
        nc.all_engine_barrier()

    TileContext._drain_and_barrier = _drain_and_barrier


def _split_excess_waits(nc, max_waits=1):
    """This walrus build accepts at most one sem wait per instruction for
    several opcodes. Hoist excess waits onto single-wait NoOps emitted just
    before the instruction on the same engine (engine streams are serial, so
    semantics are preserved)."""
    n = 0
    for fn in nc.m.functions:
        for blk in fn.blocks:
            newl = []
            for inst in blk.instructions:
                si = getattr(inst, "sync_info", None)
                if si is not None and si.on_wait is not None and len(si.on_wait) > max_waits:
                    waits = list(si.on_wait)
                    keep = waits[-max_waits:]
                    extra = waits[:-max_waits]
                    si.on_wait.clear()
                    for w in keep:
                        si.on_wait.append(w)
                    for w in extra:
                        n += 1
                        newl.append(
                            mybir.InstNoOp(
                                name=f"{inst.name}-w{n}",
                                engine=inst.engine,
                                sync_info=mybir.SyncInfo(on_wait=[w], on_update=[]),
                                bass_nofuse=True,
                            )
                        )
                newl.append(inst)
            blk.instructions[:] = newl
    return nc


def build_bass(nseg: int, split_waits: bool = True):
    """One core's program: nseg segments of SEGW class-grouped sample slots."""
    _install_tile_patch()
    slots = nseg * SEGW
    nch = slots // P                  # sub-chunks of 128 rows (2 per segment)
    ck = nseg * K                     # center rows used
    ct = (ck + P - 1) // P            # center row tiles
    ckp = ct * P                      # padded center rows
    npair = nseg // 2                 # 2-segment DMA quanta
    odd = nseg % 2

    # softmax group = 6 sub-chunks -> one PSUM bank [128, 192] f32
    GRP = max(6, (nch + 3) // 4)
    ngrp = (nch + GRP - 1) // GRP

    nc = bass.Bass()
    xg = nc.dram_tensor("xg", [slots, D], f32, kind="ExternalInput")
    cent = nc.dram_tensor("cent", [ckp, D], f32, kind="ExternalInput")
    out = nc.dram_tensor("partial", [1, 1], f32, kind="ExternalOutput")

    with TileContext(nc) as tc:
        with (
            tc.tile_pool(name="const", bufs=1) as const_pool,
            tc.tile_pool(name="persist", bufs=1) as persist,
            tc.tile_pool(name="cin", bufs=1) as cin_pool,
            tc.tile_pool(name="cnb", bufs=4) as cnb_pool,
            tc.tile_pool(name="junk", bufs=6) as junk_pool,
            tc.tile_pool(name="esb", bufs=2) as esb_pool,
            tc.tile_pool(name="tp_ps", bufs=2, space="PSUM") as tp_psum,
            tc.tile_pool(name="sc_ps", bufs=1, space="PSUM") as sc_psum,
            tc.tile_pool(name="fin_ps", bufs=1, space="PSUM") as fin_psum,
        ):
            id_f32 = const_pool.tile([P, P], f32)
            make_identity(nc, id_f32[:])
            id_bf16 = const_pool.tile([P, P], bf16)
            make_identity(nc, id_bf16[:])
            ones = const_pool.tile([P, 1], f32)
            nc.gpsimd.memset(ones[:], 1.0)

            # persistent tensors
            xf = persist.tile([P, nch * D], f32)       # sub-chunk i at cols [i*D, +D)
            xfb = persist.tile([P, nch * D], bf16)     # bf16 cast of xf (gpsimd)
            xT = persist.tile([P, DCH * slots], bf16)  # d-chunk c at cols [c*slots, +slots)
            cnT = persist.tile([P, DCH * ckp], bf16)   # d-chunk c at cols [c*ckp, +ckp)
            ssq = persist.tile([P, nch], f32)          # sum_d x^2 per sub-chunk col
            rnorm = persist.tile([P, nch], f32)        # rsqrt(ss + eps)
            zsum = persist.tile([P, nch], f32)         # softmax denominators
            nums = persist.tile([P, nch], f32)         # sum_k e_k * s_k
            c_ssr = persist.tile([P, ct], f32)         # ss_c then +eps
            c_ln = persist.tile([P, ct], f32)
            c_rn = persist.tile([P, ct], f32)          # rsqrt(ss_c + eps)

            # ---- all DMAs issued up front on the sync queue: the ring
            # streams centers then x pairs back to back.
            cfs = []
            for t in range(ct):
                cf = cin_pool.tile([P, D], f32, tag=f"cin{t}")
                cfs.append(cf)
                nc.sync.dma_start(out=cf[:], in_=cent[t * P:(t + 1) * P, :])
            for q in range(npair):
                src = xg[2 * q * SEGW:(2 * q + 2) * SEGW, :].rearrange(
                    "(s p r) d -> p s r d", s=2, p=P, r=2
                )
                dst = xf[:, q * 4 * D:(q + 1) * 4 * D].rearrange(
                    "p (s r d) -> p s r d", s=2, r=2, d=D
                )
                nc.sync.dma_start(out=dst, in_=src)
            if odd:
                src = xg[(nseg - 1) * SEGW:nseg * SEGW, :].rearrange(
                    "(p r) d -> p r d", p=P, r=2
                )
                dst = xf[:, npair * 4 * D:(npair * 4 + 2) * D].rearrange(
                    "p (r d) -> p r d", r=2, d=D
                )
                nc.sync.dma_start(out=dst, in_=src)

            # ---- centers: row sum-of-squares, rsqrt, scaled bf16 copy,
            # transpose into cnT.
            for t in range(ct):
                cjunk = junk_pool.tile([P, D], f32, tag="junkA")
                nc.scalar.activation(
                    out=cjunk[:], in_=cfs[t][:], func=AF.Square,
                    accum_out=c_ssr[:, t:t + 1],
                )
            nc.vector.tensor_scalar_add(out=c_ssr[:], in0=c_ssr[:], scalar1=1e-12)
            nc.scalar.activation(out=c_ln[:], in_=c_ssr[:], func=AF.Ln)
            nc.scalar.activation(out=c_rn[:], in_=c_ln[:], func=AF.Exp, scale=-0.5)
            for t in range(ct):
                cb = cnb_pool.tile([P, D], bf16, tag="cnb")
                nc.scalar.activation(
                    out=cb[:], in_=cfs[t][:], func=AF.Copy, scale=c_rn[:, t:t + 1],
                )
                cps = tp_psum.tile([P, D], bf16, tag="ctp", bufs=1)
                for c in range(DCH):
                    nc.tensor.transpose(
                        cps[:, c * P:(c + 1) * P], cb[:, c * P:(c + 1) * P],
                        id_bf16[:],
                    )
                nc.vector.tensor_copy(
                    out=cnT[:].rearrange("p (c n) -> p c n", c=DCH)[
                        :, :, t * P:(t + 1) * P
                    ],
                    in_=cps[:].rearrange("p (c n) -> p c n", c=DCH),
                )

            # ---- x pipeline ----
            scps = []
            egrp = []
            for g in range(ngrp):
                scp_g = sc_psum.tile([P, GRP * K], f32, tag=f"scp{g}")
                scps.append(scp_g)
                e_g = esb_pool.tile([P, GRP * K], f32, tag=f"esb{g}", bufs=1)
                ssc_g = esb_pool.tile([P, GRP * K], f32, tag=f"ssc{g}", bufs=1)
                egrp.append((e_g, ssc_g))

            mv = persist.tile([P, 2 * nch], f32)   # (mean, var) for DVE-ss subs
            mv3 = mv[:].rearrange("p (i two) -> p i two", two=2)
            ssq3 = ssq[:].rearrange("p (i one) -> p i one", one=1)
            rn3 = rnorm[:].rearrange("p (i one) -> p i one", one=1)

            def quantum(subs):
                # 1) sum of squares per sub-chunk: half ACT, half DVE
                h = len(subs) // 2
                for idx, i in enumerate(subs):
                    xfi = xf[:, i * D:(i + 1) * D]
                    if idx < h:
                        ja = junk_pool.tile([P, D], f32, tag="junkA")
                        nc.scalar.activation(
                            out=ja[:], in_=xfi, func=AF.Square,
                            accum_out=ssq[:, i:i + 1],
                        )
                    else:
                        bns = junk_pool.tile([P, 6], f32, tag="bns")
                        nc.vector.bn_stats(out=bns[:], in_=xfi)
                        nc.vector.bn_aggr(out=mv[:, 2 * i:2 * i + 2], in_=bns[:])
                # ss = D*(var + mean^2) for the DVE subs (contiguous)
                a, b = subs[h], subs[-1] + 1
                nc.vector.tensor_mul(
                    out=ssq3[:, a:b], in0=mv3[:, a:b, 0:1], in1=mv3[:, a:b, 0:1]
                )
                nc.vector.tensor_add(
                    out=ssq3[:, a:b], in0=ssq3[:, a:b], in1=mv3[:, a:b, 1:2]
                )
                nc.vector.tensor_scalar_mul(
                    out=ssq[:, a:b], in0=ssq[:, a:b], scalar1=float(D)
                )
                # 2) transpose (fp32), pack copy with bf16 cast, score matmuls
                for idx, i in enumerate(subs):
                    tps = tp_psum.tile([P, D], f32, tag="tp")
                    for c in range(DCH):
                        nc.tensor.transpose(
                            tps[:, c * P:(c + 1) * P],
                            xf[:, i * D + c * P: i * D + (c + 1) * P],
                            id_f32[:],
                        )
                    xt_dst = xT[:].rearrange("p (c n) -> p c n", c=DCH)[
                        :, :, i * P:(i + 1) * P
                    ]
                    tps_src = tps[:].rearrange("p (c n) -> p c n", c=DCH)
                    if idx % 4 == 3:
                        nc.scalar.activation(
                            out=xt_dst, in_=tps_src, func=AF.Copy,
                        )
                    else:
                        nc.vector.tensor_copy(out=xt_dst, in_=tps_src)
                    j = i // 2          # class/segment of this sub-chunk
                    g = i // GRP
                    sc = scps[g][:, (i - g * GRP) * K:(i - g * GRP + 1) * K]
                    for c in range(DCH):
                        nc.tensor.matmul(
                            sc,
                            xT[:, c * slots + i * P: c * slots + (i + 1) * P],
                            cnT[:, c * ckp + j * K: c * ckp + (j + 1) * K],
                            start=(c == 0),
                            stop=(c == DCH - 1),
                        )
                # 3) group softmax once a group's scores are complete:
                # rnorm batch (eps+ln+exp), ssc = s_raw * rnorm (broadcast),
                # e = exp(ssc), Z and num via segmented reduces.
                for i in subs:
                    g = i // GRP
                    if i != min((g + 1) * GRP, nch) - 1:
                        continue
                    c0, c1 = g * GRP, min((g + 1) * GRP, nch)
                    gw = c1 - c0
                    nc.vector.tensor_scalar_add(
                        out=ssq[:, c0:c1], in0=ssq[:, c0:c1], scalar1=1e-12
                    )
                    nc.scalar.activation(
                        out=rnorm[:, c0:c1], in_=ssq[:, c0:c1], func=AF.Ln
                    )
                    nc.scalar.activation(
                        out=rnorm[:, c0:c1], in_=rnorm[:, c0:c1], func=AF.Exp,
                        scale=-0.5,
                    )
                    ssc = egrp[g][1]
                    ssc3 = ssc[:].rearrange("p (i k) -> p i k", k=K)
                    nc.vector.tensor_mul(
                        out=ssc3[:, :gw],
                        in0=scps[g][:, :gw * K].rearrange(
                            "p (i k) -> p i k", k=K
                        ),
                        in1=rn3[:, c0:c1].broadcast_to((P, gw, K)),
                    )
                    e = egrp[g][0]
                    nc.scalar.activation(
                        out=e[:, :gw * K], in_=ssc[:, :gw * K], func=AF.Exp,
                    )
                    e3 = e[:].rearrange("p (i k) -> p i k", k=K)
                    nc.vector.tensor_reduce(
                        out=zsum[:, c0:c1], in_=e3[:, :gw],
                        axis=mybir.AxisListType.X, op=ALU.add,
                    )
                    jk = junk_pool.tile([P, GRP * K], f32, tag="jk")
                    nc.vector.tensor_mul(
                        out=jk[:, :gw * K], in0=e[:, :gw * K],
                        in1=ssc[:, :gw * K],
                    )
                    jk3 = jk[:].rearrange("p (i k) -> p i k", k=K)
                    nc.vector.tensor_reduce(
                        out=nums[:, c0:c1], in_=jk3[:, :gw],
                        axis=mybir.AxisListType.X, op=ALU.add,
                    )

            for q in range(npair):
                quantum([4 * q, 4 * q + 1, 4 * q + 2, 4 * q + 3])
            if odd:
                quantum([4 * npair, 4 * npair + 1])

            # ---- tail: t = num / Z, partial = sum over all slots ----
            nc.vector.reciprocal(out=zsum[:], in_=zsum[:])
            nc.vector.tensor_mul(out=nums[:], in0=nums[:], in1=zsum[:])
            red = persist.tile([P, 1], f32)
            nc.vector.tensor_reduce(
                out=red[:], in_=nums[:], axis=mybir.AxisListType.X, op=ALU.add,
            )
            fin = fin_psum.tile([1, 1], f32)
            nc.tensor.matmul(fin[:], red[:], ones[:], start=True, stop=True)
            osb = const_pool.tile([1, 1], f32)
            nc.scalar.copy(out=osb[:], in_=fin[:])
            nc.sync.dma_start(out=out[:], in_=osb[:])

    if split_waits:
        _split_excess_waits(nc)
    return nc


def _pack_segments(labels: np.ndarray):
    """Group sample indices by label into segments of <= SEGW, one class per
    segment; pad total segment count to a multiple of NCORES."""
    order = np.argsort(labels, kind="stable")
    sorted_lab = labels[order]
    cut = np.flatnonzero(np.diff(sorted_lab)) + 1
    starts = np.concatenate(([0], cut))
    ends = np.concatenate((cut, [len(labels)]))
    segs = []  # (class, sample_index_array)
    for s, e in zip(starts, ends):
        cls = int(sorted_lab[s])
        for o in range(s, e, SEGW):
            segs.append((cls, order[o:min(o + SEGW, e)]))
    while len(segs) % NCORES != 0:
        segs.append((0, np.empty(0, dtype=np.int64)))
    return segs


def kernel(x: np.ndarray, labels: np.ndarray, centers: np.ndarray) -> np.ndarray:
    x = np.ascontiguousarray(x, dtype=np.float32)
    labels = np.asarray(labels)
    centers = np.ascontiguousarray(centers, dtype=np.float32)
    nb, d = x.shape
    ncls, k, _ = centers.shape
    assert (nb, d, k) == (B, D, K)

    segs = _pack_segments(labels)
    nseg_total = len(segs)
    nseg = nseg_total // NCORES
    slots = nseg * SEGW
    ck = nseg * K
    ckp = ((ck + P - 1) // P) * P

    in_maps = []
    for core in range(NCORES):
        xg = np.zeros((slots, d), dtype=np.float32)
        cent = np.zeros((ckp, d), dtype=np.float32)
        for jj in range(nseg):
            cls, idx = segs[core * nseg + jj]
            if len(idx):
                xg[jj * SEGW: jj * SEGW + len(idx)] = x[idx]
            cent[jj * K:(jj + 1) * K] = centers[cls]
        in_maps.append({"xg": xg, "cent": cent})

    nc = build_bass(nseg)
    res = run_bass_kernel_spmd(nc, in_maps, core_ids=list(range(NCORES)))
    total = sum(float(r["partial"][0, 0]) for r in res.results)
    return np.float32(1.0 - total / nb)


# revision 24
# speedup vs baseline: 1.9517x; 1.1769x over previous
"""Trainium2 Bass kernel for CenterWoParamMultiCosineSoftmaxLoss.

loss = mean_b sum_k softmax_k(2 - dst_bk) * dst_bk,
  dst_bk = 1 - <x_b/||x_b||, c_{l_b,k}/||c_{l_b,k}||>

Identities: softmax(2-dst) = softmax(s) (shift invariance, s = cosine);
per_sample = 1 - sum_k p_k s_k.

Distribution (zero padding): samples are SORTED by label on the host and
split into 8 equal contiguous slices of 2048 rows - every core processes
16 sub-chunks of 128 rows with NO pad slots. A 256-row window (one DMA
pair) spans at most W classes (W<=3 for ~uniform labels since every class
has >=128 members); each sub-chunk's scores are computed against all W
window classes (W*K columns) and wrong-class columns are killed by adding
-3e4 inside the same PSUM accumulation via one rank-W matmul
(U[c,slot] x V[c,k] with U = per-slot class indicators DMA'd as data,
V = constant block pattern), so exp() zeroes them exactly.

Per core: x pair-DMAs land as [128, 4KB] lines (rows 2p, 2p+1 per
partition); per sub-chunk: sum-of-squares (ACT Square+accum / DVE
bn_stats split), 4 fp32 PE transposes, pair-batched PSUM->SBUF cast copy
to bf16 xT, 5 accumulating bf16 score matmuls (4 d-chunks + mask);
per group of 4 sub-chunks: batched rsqrt, ssc = s*rnorm via broadcast
multiply, one exp, segmented Z/num reduces; batched tail reduce and a
ones-matmul for the cross-partition sum. Centers arrive per-window
duplicated (W classes x 32 rows per pair), are normalized on device and
transposed into a per-pair cnT table; all DMAs are issued up front on
the sync queue.
"""

import sys

for _p in ("/opt/trn_rl_repo", "/root/.axon_site/_ro/trn_rl_repo"):
    if _p not in sys.path:
        sys.path.append(_p)

import numpy as np

import concourse.bass as bass
import concourse.mybir as mybir
from concourse.tile import TileContext
from concourse.masks import make_identity
from concourse.bass_utils import run_bass_kernel_spmd
from concourse.vector_clock import ScopedClock

B, D, C, K = 16384, 512, 90, 32
NCORES = 8
P = 128
DCH = D // P
CORE_ROWS = B // NCORES          # 2048
NCH = CORE_ROWS // P             # 16 sub-chunks
NPAIR = NCH // 2                 # 8 pair quanta (256 rows each)
GRP = 4                          # sub-chunks per softmax group
NGRP = NCH // GRP                # 4 groups
f32 = mybir.dt.float32
bf16 = mybir.dt.bfloat16
AF = mybir.ActivationFunctionType
ALU = mybir.AluOpType
NEG = -30000.0                   # mask bias (survives rnorm scaling)

_tile_patched = False


def _install_tile_patch():
    """This walrus build allows only one sem wait on TPB_CTRL-lowered
    instructions (Drain / sync-NoOp). Tile's tail drain attaches one wait per
    live processor clock; split them into a chain of single-wait NoOps."""
    global _tile_patched
    if _tile_patched:
        return
    _tile_patched = True

    def _drain_and_barrier(self, tick_clock, wait_clock):
        nc = self.nc
        probe = nc.sync.nop(nofuse=True)
        wait_clock.add_sem_waits(
            probe.ins, ScopedClock({None: tick_clock.global_clock})
        )
        si = probe.ins.sync_info
        if si is not None and len(si.on_wait) > 1:
            waits = list(si.on_wait)
            si.on_wait.clear()
            si.on_wait.append(waits[0])
            for w in waits[1:]:
                n2 = nc.sync.nop(nofuse=True)
                if n2.ins.sync_info is None:
                    n2.ins.sync_info = mybir.SyncInfo(on_wait=[w], on_update=[])
                else:
                    n2.ins.sync_info.on_wait.append(w)
        nc.sync.drain()
        nc.all_engine_barrier()
        assert self.sems is not None
        popped = nc._tile_sem_poison_stack.pop()
        assert popped is self._sem_poison
        nc.clear_and_free_semaphores(list(self.sems.allocated().values()))
        nc.all_engine_barrier()

    TileContext._drain_and_barrier = _drain_and_barrier


def _split_excess_waits(nc, max_waits=1):
    """This walrus build accepts at most one sem wait per instruction for
    several opcodes; hoist excess waits onto single-wait NoOps."""
    n = 0
    for fn in nc.m.functions:
        for blk in fn.blocks:
            newl = []
            for inst in blk.instructions:
                si = getattr(inst, "sync_info", None)
                if si is not None and si.on_wait is not None and len(si.on_wait) > max_waits:
                    waits = list(si.on_wait)
                    keep = waits[-max_waits:]
                    extra = waits[:-max_waits]
                    si.on_wait.clear()
                    for w in keep:
                        si.on_wait.append(w)
                    for w in extra:
                        n += 1
                        newl.append(
                            mybir.InstNoOp(
                                name=f"{inst.name}-w{n}",
                                engine=inst.engine,
                                sync_info=mybir.SyncInfo(on_wait=[w], on_update=[]),
                                bass_nofuse=True,
                            )
                        )
                newl.append(inst)
            blk.instructions[:] = newl
    return nc


def build_bass(W: int, split_waits: bool = True):
    """One core's program. W = max classes per 256-row window."""
    _install_tile_patch()
    wk = W * K                        # score columns per sub-chunk
    crows = NPAIR * wk                # duplicated center rows (8 * W * 32)
    ct = (crows + P - 1) // P         # duplicated center tiles
    crp = ct * P

    nc = bass.Bass()
    xg = nc.dram_tensor("xg", [CORE_ROWS, D], f32, kind="ExternalInput")
    cent = nc.dram_tensor("cent", [crp, D], f32, kind="ExternalInput")
    ut = nc.dram_tensor("ut", [W, NCH * P + wk], f32, kind="ExternalInput")
    out = nc.dram_tensor("partial", [1, 1], f32, kind="ExternalOutput")

    with TileContext(nc) as tc:
        with (
            tc.tile_pool(name="const", bufs=1) as const_pool,
            tc.tile_pool(name="persist", bufs=1) as persist,
            tc.tile_pool(name="cin", bufs=1) as cin_pool,
            tc.tile_pool(name="cnb", bufs=3) as cnb_pool,
            tc.tile_pool(name="junk", bufs=4) as junk_pool,
            tc.tile_pool(name="esb", bufs=1) as esb_pool,
            tc.tile_pool(name="tp_ps", bufs=2, space="PSUM") as tp_psum,
            tc.tile_pool(name="sc_ps", bufs=1, space="PSUM") as sc_psum,
        ):
            id_f32 = const_pool.tile([P, P], f32)
            make_identity(nc, id_f32[:])
            id_bf16 = const_pool.tile([P, P], bf16)
            make_identity(nc, id_bf16[:])
            ones = const_pool.tile([P, 1], f32)
            nc.gpsimd.memset(ones[:], 1.0)

            # persistent tensors
            xf = persist.tile([P, NCH * D], f32)
            xT = persist.tile([P, DCH * CORE_ROWS], bf16)
            uf = persist.tile([W, NCH * P + wk], f32)
            ub = persist.tile([W, NCH * P + wk], bf16)
            # mask pattern V[c, k] = NEG where k's class-block != c (last wk
            # columns of the DMA'd ut tensor)
            vpat = ub[:, NCH * P:NCH * P + wk]
            cnT = persist.tile([P, DCH * crp], bf16)
            ssq = persist.tile([P, NCH], f32)
            rnorm = persist.tile([P, NCH], f32)
            zsum = persist.tile([P, NCH], f32)
            nums = persist.tile([P, NCH], f32)
            mv = persist.tile([P, 2 * NCH], f32)
            c_ssr = persist.tile([P, ct], f32)
            c_rn = persist.tile([P, ct], f32)
            mv3 = mv[:].rearrange("p (i two) -> p i two", two=2)
            ssq3 = ssq[:].rearrange("p (i one) -> p i one", one=1)
            rn3 = rnorm[:].rearrange("p (i one) -> p i one", one=1)

            # ---- all input DMAs up front on the sync queue ----
            nc.sync.dma_start(out=uf[:], in_=ut[:, :])
            cfs = []
            for t in range(ct):
                cf = cin_pool.tile([P, D], f32, tag=f"cin{t}")
                cfs.append(cf)
                nc.sync.dma_start(out=cf[:], in_=cent[t * P:(t + 1) * P, :])
            for q in range(NPAIR):
                src = xg[2 * q * P:2 * (q + 1) * P, :].rearrange(
                    "(p r) d -> p r d", p=P, r=2
                )
                dst = xf[:, q * 2 * D:(q + 1) * 2 * D].rearrange(
                    "p (r d) -> p r d", r=2, d=D
                )
                nc.sync.dma_start(out=dst, in_=src)

            # U as bf16 for the mask matmuls
            nc.vector.tensor_copy(out=ub[:], in_=uf[:])

            # ---- centers: normalize + transpose into cnT ----
            for t in range(ct):
                if t % 2 == 0:
                    cjunk = junk_pool.tile([P, D], f32, tag="junkA")
                    nc.scalar.activation(
                        out=cjunk[:], in_=cfs[t][:], func=AF.Square,
                        accum_out=c_ssr[:, t:t + 1],
                    )
                else:
                    bns = junk_pool.tile([P, 6], f32, tag="bns")
                    nc.vector.bn_stats(out=bns[:], in_=cfs[t][:])
                    nc.vector.bn_aggr(out=mv[:, 0:2], in_=bns[:])
                    nc.vector.tensor_mul(
                        out=c_ssr[:, t:t + 1], in0=mv[:, 0:1], in1=mv[:, 0:1]
                    )
                    nc.vector.tensor_add(
                        out=c_ssr[:, t:t + 1], in0=c_ssr[:, t:t + 1],
                        in1=mv[:, 1:2],
                    )
                    nc.vector.tensor_scalar_mul(
                        out=c_ssr[:, t:t + 1], in0=c_ssr[:, t:t + 1],
                        scalar1=float(D),
                    )
            nc.vector.tensor_scalar_add(out=c_ssr[:], in0=c_ssr[:], scalar1=1e-12)
            nc.scalar.activation(out=c_rn[:], in_=c_ssr[:], func=AF.Ln)
            nc.scalar.activation(out=c_rn[:], in_=c_rn[:], func=AF.Exp, scale=-0.5)
            for t in range(ct):
                cb = cnb_pool.tile([P, D], bf16, tag="cnb")
                nc.scalar.activation(
                    out=cb[:], in_=cfs[t][:], func=AF.Copy, scale=c_rn[:, t:t + 1],
                )
                cps = tp_psum.tile([P, D], bf16, tag="tp")
                for c in range(DCH):
                    nc.tensor.transpose(
                        cps[:, c * P:(c + 1) * P], cb[:, c * P:(c + 1) * P],
                        id_bf16[:],
                    )
                nc.vector.tensor_copy(
                    out=cnT[:].rearrange("p (c n) -> p c n", c=DCH)[
                        :, :, t * P:(t + 1) * P
                    ],
                    in_=cps[:].rearrange("p (c n) -> p c n", c=DCH),
                )

            # ---- x pipeline ----
            scps = []
            egrp = []
            for g in range(NGRP):
                scp_g = sc_psum.tile([P, GRP * wk], f32, tag=f"scp{g}")
                scps.append(scp_g)
                e_g = esb_pool.tile([P, GRP * wk], f32, tag=f"esb{g}", bufs=1)
                ssc_g = esb_pool.tile([P, GRP * wk], f32, tag=f"ssc{g}", bufs=1)
                egrp.append((e_g, ssc_g))

            for q in range(NPAIR):
                i0, i1 = 2 * q, 2 * q + 1
                # 1) sum of squares: i0 on ACT, i1 on DVE
                ja = junk_pool.tile([P, D], f32, tag="junkA")
                nc.scalar.activation(
                    out=ja[:], in_=xf[:, i0 * D:(i0 + 1) * D], func=AF.Square,
                    accum_out=ssq[:, i0:i0 + 1],
                )
                bns = junk_pool.tile([P, 6], f32, tag="bns")
                nc.vector.bn_stats(out=bns[:], in_=xf[:, i1 * D:(i1 + 1) * D])
                nc.vector.bn_aggr(out=mv[:, 2 * i1:2 * i1 + 2], in_=bns[:])
                nc.vector.tensor_mul(
                    out=ssq3[:, i1:i1 + 1], in0=mv3[:, i1:i1 + 1, 0:1],
                    in1=mv3[:, i1:i1 + 1, 0:1],
                )
                nc.vector.tensor_add(
                    out=ssq3[:, i1:i1 + 1], in0=ssq3[:, i1:i1 + 1],
                    in1=mv3[:, i1:i1 + 1, 1:2],
                )
                nc.vector.tensor_scalar_mul(
                    out=ssq[:, i1:i1 + 1], in0=ssq[:, i1:i1 + 1],
                    scalar1=float(D),
                )
                # 2) transposes into one 2-bank PSUM tile, one pair copy
                tps = tp_psum.tile([P, 2 * D], f32, tag="tp")
                for idx, i in enumerate((i0, i1)):
                    for c in range(DCH):
                        nc.tensor.transpose(
                            tps[:, idx * D + c * P: idx * D + (c + 1) * P],
                            xf[:, i * D + c * P: i * D + (c + 1) * P],
                            id_f32[:],
                        )
                # copy [p][i2][c4][128] -> xT [p][c4][slot 256]
                tps_src = tps[:].rearrange("p (i c n) -> p i c n", i=2, c=DCH)
                xt_dst = xT[:].rearrange("p (c n) -> p c n", c=DCH)[
                    :, :, q * 2 * P:(q + 1) * 2 * P
                ].rearrange("p c (i n) -> p i c n", i=2)
                if q % 2 == 0:
                    nc.scalar.activation(out=xt_dst, in_=tps_src, func=AF.Copy)
                else:
                    nc.vector.tensor_copy(out=xt_dst, in_=tps_src)
                # 3) scores: 4 d-chunk matmuls + mask matmul per sub-chunk
                for i in (i0, i1):
                    g = i // GRP
                    sc = scps[g][:, (i - g * GRP) * wk:(i - g * GRP + 1) * wk]
                    for c in range(DCH):
                        nc.tensor.matmul(
                            sc,
                            xT[:, c * CORE_ROWS + i * P: c * CORE_ROWS + (i + 1) * P],
                            cnT[:, c * crp + q * wk: c * crp + (q + 1) * wk],
                            start=(c == 0),
                            stop=False,
                        )
                    nc.tensor.matmul(
                        sc,
                        ub[:, i * P:(i + 1) * P],
                        vpat,
                        start=False,
                        stop=True,
                    )
                # 4) group softmax after the group's last sub-chunk
                if i1 % GRP == GRP - 1:
                    g = i1 // GRP
                    c0, c1 = g * GRP, (g + 1) * GRP
                    nc.vector.tensor_scalar_add(
                        out=ssq[:, c0:c1], in0=ssq[:, c0:c1], scalar1=1e-12
                    )
                    nc.scalar.activation(
                        out=rnorm[:, c0:c1], in_=ssq[:, c0:c1], func=AF.Ln
                    )
                    nc.scalar.activation(
                        out=rnorm[:, c0:c1], in_=rnorm[:, c0:c1], func=AF.Exp,
                        scale=-0.5,
                    )
                    e, ssc = egrp[g]
                    ssc3 = ssc[:].rearrange("p (i k) -> p i k", k=wk)
                    nc.vector.tensor_mul(
                        out=ssc3,
                        in0=scps[g][:].rearrange("p (i k) -> p i k", k=wk),
                        in1=rn3[:, c0:c1].broadcast_to((P, GRP, wk)),
                    )
                    nc.scalar.activation(out=e[:], in_=ssc[:], func=AF.Exp)
                    e3 = e[:].rearrange("p (i k) -> p i k", k=wk)
                    nc.vector.tensor_reduce(
                        out=zsum[:, c0:c1], in_=e3,
                        axis=mybir.AxisListType.X, op=ALU.add,
                    )
                    jk = junk_pool.tile([P, GRP * wk], f32, tag="jk")
                    nc.vector.tensor_mul(out=jk[:], in0=e[:], in1=ssc[:])
                    jk3 = jk[:].rearrange("p (i k) -> p i k", k=wk)
                    nc.vector.tensor_reduce(
                        out=nums[:, c0:c1], in_=jk3,
                        axis=mybir.AxisListType.X, op=ALU.add,
                    )

            # ---- tail: t = num / Z, partial = sum over all slots ----
            nc.vector.reciprocal(out=zsum[:], in_=zsum[:])
            nc.vector.tensor_mul(out=nums[:], in0=nums[:], in1=zsum[:])
            red = persist.tile([P, 1], f32)
            nc.vector.tensor_reduce(
                out=red[:], in_=nums[:], axis=mybir.AxisListType.X, op=ALU.add,
            )
            fin = sc_psum.tile([1, 1], f32, tag="scp0")
            nc.tensor.matmul(fin[:], red[:], ones[:], start=True, stop=True)
            osb = const_pool.tile([1, 1], f32)
            nc.scalar.copy(out=osb[:], in_=fin[:])
            nc.sync.dma_start(out=out[:], in_=osb[:])

    if split_waits:
        _split_excess_waits(nc)
    return nc


def _pack_sorted(labels: np.ndarray):
    """Sort rows by label; per core, per 256-row window compute the class
    window (padded to global W) and per-slot class indicators."""
    order = np.argsort(labels, kind="stable")
    lab = np.asarray(labels)[order]
    wins = []   # [core][pair] -> list of classes
    W = 1
    for core in range(NCORES):
        rows = lab[core * CORE_ROWS:(core + 1) * CORE_ROWS]
        cw = []
        for q in range(NPAIR):
            wlab = rows[q * 2 * P:(q + 1) * 2 * P]
            cls = sorted(set(int(v) for v in wlab))
            W = max(W, len(cls))
            cw.append(cls)
        wins.append(cw)
    return order, wins, W


def kernel(x: np.ndarray, labels: np.ndarray, centers: np.ndarray) -> np.ndarray:
    x = np.ascontiguousarray(x, dtype=np.float32)
    labels = np.asarray(labels)
    centers = np.ascontiguousarray(centers, dtype=np.float32)
    nb, d = x.shape
    ncls, k, _ = centers.shape
    assert (nb, d, k) == (B, D, K)

    order, wins, W = _pack_sorted(labels)
    lab_sorted = labels[order]
    wk = W * K
    crows = NPAIR * wk
    crp = ((crows + P - 1) // P) * P

    in_maps = []
    for core in range(NCORES):
        rows = order[core * CORE_ROWS:(core + 1) * CORE_ROWS]
        rl = lab_sorted[core * CORE_ROWS:(core + 1) * CORE_ROWS]
        xg = x[rows]
        cent = np.zeros((crp, d), dtype=np.float32)
        uts = np.zeros((W, NCH * P + wk), dtype=np.float32)
        for c in range(W):
            uts[c, NCH * P:] = NEG
            uts[c, NCH * P + c * K: NCH * P + (c + 1) * K] = 0.0
        for q in range(NPAIR):
            cls = wins[core][q]
            for c, cl in enumerate(cls):
                cent[q * wk + c * K: q * wk + (c + 1) * K] = centers[cl]
            # per-slot indicators: sub-chunk i=2q+r, slot p = row 2p+r
            wl = rl[q * 2 * P:(q + 1) * 2 * P]
            for r in range(2):
                i = 2 * q + r
                sl = wl[np.arange(P) * 2 + r]        # labels per slot
                for c, cl in enumerate(cls):
                    uts[c, i * P:(i + 1) * P] = (sl == cl).astype(np.float32)
        in_maps.append({"xg": xg, "cent": cent, "ut": uts})

    nc = build_bass(W)
    res = run_bass_kernel_spmd(nc, in_maps, core_ids=list(range(NCORES)))
    total = sum(float(r["partial"][0, 0]) for r in res.results)
    return np.float32(1.0 - total / nb)
